# revision 31
# baseline (speedup 1.0000x reference)
"""Trainium2 Bass kernel for nn_CapsuleLayer (dynamic routing).

Math (per independent column c=(k,b,o), vector p = pred[k,b,:,o] of length N):
    logits stay proportional to p:  logits_t = p * V_t  with scalar V_t.
    iter 1: c uniform -> s1 = mean_n(p);  v1 = squash(s1); V1 = v1
    iter t: Z = sum_n exp(V*p), Y = sum_n p*exp(V*p), s = Y/Z,
            v = squash(s) = s*|s|/(1+s^2), V += v
    output = v from the last iteration.

Sharding: data-parallel over batch (32 of 256 per core, 8 cores).

Per-core device pipeline:
  pred is computed by PE as  Wr[ng].T @ xbd[bg,ng]  where xbd is a
  host-built block-diagonal slab of x (contraction = 16 n-values x 8 cin),
  so PSUM comes out column-major: rows=(k,o), free=(nl, bl).
    - "T1" rows = ko 0..127 (k0..7)  -> A1[bg] tiles [128, 8*1152] fp16
    - "A2" rows = ko 128..159 (k8,k9), bl-major: partitions (q=bl%4, ko2),
      free (bg, blh=bl//4, ng, nl) -> A2 tile [128, 9216] fp16.  Each
      route instruction then covers a full [128, 1152] slab with a
      per-partition V scale, so no j-reduce fixups are needed.
  Routing per (column-set, iteration) triple: ScalarE exp (per-partition
  scale=V, optionally fused accum_out=Z), Z otherwise via DVE
  affine_mul_reduce (out=(p*0+1)*e in-place, accum=Z), and
  Y = sum p*e via DVE affine_mul_reduce (2x mode) or GpSimd
  scalar_tensor_tensor -- statically load-balanced across engines.
"""

import sys

sys.path.insert(0, "/opt/trn_rl_repo")

from contextlib import ExitStack

import numpy as np

import concourse.bass as bass  # noqa: F401
import concourse.bacc as bacc
import concourse.tile as tile
from concourse import mybir
from concourse.bass_utils import run_bass_kernel_spmd

# ---- problem constants (hardcoded per harness contract) ----
B, N, CIN = 256, 1152, 8
K, O = 10, 16
KO = K * O            # 160
NCORES = 8
BSH = B // NCORES     # 32 batch per core
BG, BL = 4, 8         # batch groups x lanes (BSH = BG*BL)
NG, NL = 72, 16       # n-groups x n-lanes (N = NG*NL)
F32 = mybir.dt.float32
F16 = mybir.dt.float16

# ---- engine load-balance knobs ----
ZD_MOD = 0            # Z on DVE amr when idx % ZD_MOD == 0 (0 = always Act)
EVAC_ACT_MOD = (0, 2, 2, 2)   # per-bg: evac on Act every Nth (0 = never)

_cache = {}


# ----------------------------------------------------------------------------
# host-side input prep
# ----------------------------------------------------------------------------
def _prep_shared(w):
    # Wr[ng, 8*nl+i, 16*k+o] = w[k, 16*ng+nl, i, o]; ship partition-major
    wr = np.transpose(
        w.reshape(K, NG, NL, CIN, O), (1, 2, 3, 0, 4)
    ).reshape(NG, 128, KO).astype(np.float16)
    wr = np.ascontiguousarray(np.transpose(wr, (1, 0, 2)).reshape(128, NG * KO))
    ident32 = np.eye(32, dtype=np.float32)
    return wr, ident32


def _prep_core_inputs(x, w):
    wr, ident32 = _prep_shared(w)
    in_maps = []
    for c in range(NCORES):
        xc = x[c * BSH:(c + 1) * BSH]                          # [32, N, CIN]
        # xs[ng, 8*nl+i, b] = xc[b, 16*ng+nl, i]
        xs = np.transpose(
            xc.reshape(BSH, NG, NL, CIN), (1, 2, 3, 0)
        ).reshape(NG, 128, BSH).astype(np.float16)
        xs = np.ascontiguousarray(
            np.transpose(xs, (1, 0, 2)).reshape(128, NG * BSH))
        # xbd[bg, ng, (nl',i), (nl,bl)] = xc[8bg+bl, 16ng+nl, i] * (nl==nl')
        xbd = np.zeros((BG, NG, NL, CIN, NL, BL), dtype=np.float16)
        xs5 = np.transpose(
            xc.reshape(BG, BL, NG, NL, CIN), (0, 2, 3, 4, 1)
        ).astype(np.float16)                                   # [bg,ng,nl,i,bl]
        for r in range(NL):
            xbd[:, :, r, :, r, :] = xs5[:, :, r, :, :]
        xbd = np.ascontiguousarray(
            np.transpose(xbd.reshape(BG, NG, 128, 128),
                         (0, 2, 1, 3)).reshape(BG, 128, NG * 128))
        in_maps.append({
            "xbd": xbd, "xs": xs, "wr": wr, "ident32": ident32,
        })
    return in_maps


# ----------------------------------------------------------------------------
# device program
# ----------------------------------------------------------------------------
def _interleave(*gens):
    gens = list(gens)
    while gens:
        nxt = []
        for g in gens:
            try:
                next(g)
                nxt.append(g)
            except StopIteration:
                pass
        gens = nxt


def _build_program(T):
    nc = bacc.Bacc("TRN2", target_bir_lowering=False, debug=False,
                   enable_asserts=False)

    xbd_d = nc.dram_tensor("xbd", [BG, 128, NG * 128], F16, kind="ExternalInput").ap()
    xs_d = nc.dram_tensor("xs", [128, NG * BSH], F16, kind="ExternalInput").ap()
    wr_d = nc.dram_tensor("wr", [128, NG * KO], F16, kind="ExternalInput").ap()
    id_d = nc.dram_tensor("ident32", [32, 32], F32, kind="ExternalInput").ap()
    out1_d = nc.dram_tensor("out1", [BG, 128, BL], F32, kind="ExternalOutput").ap()
    out2_d = nc.dram_tensor("out2", [128, 8], F32, kind="ExternalOutput").ap()

    mult = mybir.AluOpType.mult
    add = mybir.AluOpType.add
    EXP = mybir.ActivationFunctionType.Exp
    smp_bufs = max(14, 8 * (T - 1) + 2)

    with tile.TileContext(nc) as tc, ExitStack() as ctx:
        consts = ctx.enter_context(tc.tile_pool(name="consts", bufs=1))
        a1p = ctx.enter_context(tc.tile_pool(name="a1", bufs=3))
        a2p = ctx.enter_context(tc.tile_pool(name="a2", bufs=1))
        xbdp = ctx.enter_context(tc.tile_pool(name="xbd", bufs=2))
        ep = ctx.enter_context(tc.tile_pool(name="e", bufs=8))
        smp = ctx.enter_context(tc.tile_pool(name="sm", bufs=smp_bufs))
        psA = ctx.enter_context(tc.tile_pool(name="psA", bufs=2, space="PSUM"))
        psB = ctx.enter_context(tc.tile_pool(name="psB", bufs=2, space="PSUM"))
        psM = ctx.enter_context(tc.tile_pool(name="psM", bufs=1, space="PSUM"))
        psT = ctx.enter_context(tc.tile_pool(name="psT", bufs=1, space="PSUM"))

        # ---- resident inputs ----
        # bg0's xbd slab is the first matmul dependency: interleave its
        # chunks ahead of/between the weight chunks in DMA emission order.
        wrs = consts.tile([128, NG * KO], F16, tag="wrs", name="wrs")
        xsal = consts.tile([128, NG * BSH], F16, tag="xsal", name="xsal")
        xbt0 = xbdp.tile([128, NG * 128], F16, tag="xbd", name="xbd")
        WCH = 4
        for ch in range(WCH):
            if ch < 3:
                c0, c1 = ch * NG // 3, (ch + 1) * NG // 3
                nc.sync.dma_start(xbt0[:, c0 * 128:c1 * 128],
                                  xbd_d[0, :, c0 * 128:c1 * 128])
            g0, g1 = ch * NG // WCH, (ch + 1) * NG // WCH
            nc.sync.dma_start(wrs[:, g0 * KO:g1 * KO], wr_d[:, g0 * KO:g1 * KO])
            nc.sync.dma_start(xsal[:, g0 * BSH:g1 * BSH],
                              xs_d[:, g0 * BSH:g1 * BSH])
        id32 = consts.tile([32, 32], F32, tag="id32", name="id32")
        nc.sync.dma_start(id32[:], id_d)
        ones1 = consts.tile([128, 1], F32, tag="ones1", name="ones1")
        nc.vector.memset(ones1[:], 1.0)

        # A2 accumulator: rows (q=bl%4, ko2); free (bg, blh=bl//4, ng, nl)
        a2t = a2p.tile([128, BG * 2 * NG * NL], F16, tag="a2", name="a2")
        a2v = a2t[:].rearrange("p (G h g l) -> p G h g l",
                               G=BG, h=2, g=NG, l=NL)

        st = {}
        a1t_of = {}
        evac_tgl = [0]
        tri = [0]

        def evac_copy(dst, src, bg=0):
            mod = EVAC_ACT_MOD[bg]
            if mod and evac_tgl[0] % mod == 0:
                nc.scalar.copy(dst, src)
            else:
                nc.vector.tensor_copy(dst, src)
            evac_tgl[0] += 1

        def zy_triple(p_sl, Vscale, Z_ap, Y_ap):
            """exp + Z + Y for one [128, N] column-set; engines per knobs."""
            i = tri[0]
            tri[0] += 1
            e = ep.tile([128, N], F16, tag="e", name="e")
            z_act = (ZD_MOD == 0) or (i % ZD_MOD) != 0
            if z_act:
                nc.scalar.activation(e[:], p_sl, EXP, scale=Vscale,
                                     accum_out=Z_ap)
            else:
                nc.scalar.activation(e[:], p_sl, EXP, scale=Vscale)
                nc.vector.affine_mul_reduce(
                    out=e[:], accum_out=Z_ap, in0=p_sl, in1=e[:],
                    scale=0.0, bias=1.0)
            nc.vector.affine_mul_reduce(
                out=e[:], accum_out=Y_ap, in0=e[:], in1=p_sl,
                scale=1.0, bias=0.0)

        def squash(s_ap, P, W):
            """v = s*|s|/(1+s*s) as a fresh [P, W] f32 tile"""
            n2 = smp.tile([P, W], F32, tag=f"sq_n2_{P}_{W}", name=f"sq_n2_{P}_{W}")
            nc.vector.tensor_tensor(n2[:], s_ap, s_ap, mult)
            d = smp.tile([P, W], F32, tag=f"sq_d_{P}_{W}", name=f"sq_d_{P}_{W}")
            nc.vector.tensor_scalar_add(d[:], n2[:], 1.0)
            r = smp.tile([P, W], F32, tag=f"sq_r_{P}_{W}", name=f"sq_r_{P}_{W}")
            nc.vector.reciprocal(r[:], d[:])
            a = smp.tile([P, W], F32, tag=f"sq_a_{P}_{W}", name=f"sq_a_{P}_{W}")
            nc.scalar.activation(a[:], s_ap, mybir.ActivationFunctionType.Abs)
            t = smp.tile([P, W], F32, tag=f"sq_t_{P}_{W}", name=f"sq_t_{P}_{W}")
            nc.vector.tensor_tensor(t[:], s_ap, a[:], mult)
            v = smp.tile([P, W], F32, tag=f"sq_v_{P}_{W}", name=f"sq_v_{P}_{W}")
            nc.vector.tensor_tensor(v[:], t[:], r[:], mult)
            return v

        # ------------------------------------------------------------------
        def gen_phase(bg):
            a1t = a1p.tile([128, BL * N], F16, tag="a1", name="a1")
            st[("a1", bg)] = a1t
            a1t_of[bg] = a1t
            a1v = a1t[:].rearrange("p (b g l) -> p g l b", b=BL, g=NG, l=NL)
            if bg == 0:
                xbt = xbt0
            else:
                xbt = xbdp.tile([128, NG * 128], F16, tag="xbd", name="xbd")
                for ch in range(3):
                    c0, c1 = ch * NG // 3, (ch + 1) * NG // 3
                    nc.sync.dma_start(xbt[:, c0 * 128:c1 * 128],
                                      xbd_d[bg, :, c0 * 128:c1 * 128])
            # xbd columns (nl, bl) split as (l, h=bl//4, q=bl%4)
            xbq = xbt[:].rearrange("p (g l h q) -> p g q l h",
                                   g=NG, l=NL, h=2, q=4)
            pa = None
            pb = None
            pa_q0 = 0
            pb_q0 = 0
            NQ = NG // 4
            for Qn in range(NQ):              # 18 blocks of 4 ng
                if Qn % 2 == 0:
                    pa = psA.tile([128, 1024], F32, tag="psA", name="psA")
                    pa_q0 = Qn
                    pb = psB.tile([128, 256], F32, tag="psB", name="psB")
                    pb_q0 = Qn
                for j in range(4):
                    ng = 4 * Qn + j
                    w0 = wrs[:, ng * KO:ng * KO + 128]
                    w1 = wrs[:, ng * KO + 128:ng * KO + KO]
                    rhs = xbt[:, ng * 128:(ng + 1) * 128]
                    jj = (Qn - pa_q0) * 4 + j
                    nc.tensor.matmul(pa[:, jj * 128:(jj + 1) * 128],
                                     w0, rhs, start=True, stop=True)
                    ngi = (Qn - pb_q0) * 4 + j
                    for q in range(4):
                        nc.tensor.matmul(
                            pb[32 * q:32 * q + 32, ngi * 32:(ngi + 1) * 32],
                            w1, xbq[:, ng, q], start=True, stop=True,
                            tile_position=(0, 32 * q))
                    if bg == 0:
                        nc.tensor.matmul(
                            st["m1ps"][:],
                            xsal[:, ng * BSH:(ng + 1) * BSH],
                            wrs[:, ng * KO:(ng + 1) * KO],
                            start=(ng == 0), stop=(ng == NG - 1))
                # evacuate psA -> A1[bg]; dst/src iteration order = (g, l, b)
                if Qn % 2 == 1:
                    evac_copy(a1v[:, 4 * pa_q0:4 * pa_q0 + 8, :, :],
                              pa[:].rearrange("p (g l b) -> p g l b",
                                              g=8, l=NL, b=BL), bg=bg)
                    ng0 = 4 * pb_q0
                    evac_copy(
                        a2v[:, bg, :, ng0:ng0 + 8, :],
                        pb[:].rearrange("p (g l h) -> p h g l",
                                        g=8, l=NL, h=2), bg=bg)
                yield

        # ------------------------------------------------------------------
        def m1_finalize():
            m1s = smp.tile([32, KO], F32, tag="m1s", name="m1s")
            nc.vector.tensor_copy(m1s[:], st["m1ps"][:])
            t1 = psT.tile([128, 32], F32, tag="psT", name="psT")
            nc.tensor.transpose(t1[:], m1s[:, 0:128], id32[:])
            s1 = smp.tile([128, 32], F32, tag="s1t1", name="s1t1")
            nc.vector.tensor_scalar_mul(s1[:], t1[:], 1.0 / N)
            v1 = squash(s1[:], 128, 32)
            st["V_t1_1"] = v1              # [128=(ko), 32=(bg,bl)]
            t2 = psT.tile([128, 32], F32, tag="psT", name="psT")
            nc.tensor.transpose(t2[:32, :], m1s[:, 128:KO], id32[:])
            s1a = smp.tile([32, 32], F32, tag="s1a2", name="s1a2")
            nc.vector.tensor_scalar_mul(s1a[:], t2[:32, :], 1.0 / N)
            v1a = squash(s1a[:], 32, 32)   # [32=(ko2), 32=(bg,bl)]
            yield
            # bl-major replication: Vb[(q,ko2), (bg,blh)] = v1a[ko2, 8bg+4blh+q]
            v1aq = v1a[:].rearrange("k (g h q) -> k q g h", g=BG, h=2, q=4)
            vb = psT.tile([128, 32], F32, tag="psT", name="psT")
            for q in range(4):
                nc.tensor.matmul(vb[32 * q:32 * q + 32, 0:8], id32[:],
                                 v1aq[:, q], start=True, stop=True,
                                 tile_position=(0, 32 * q))
            vbig = smp.tile([128, 8], F32, tag="va2bm", name="va2bm")
            nc.vector.tensor_copy(vbig[:], vb[:, 0:8])
            st["Va2_1"] = vbig             # [128=(q,ko2), 8=(bg,blh)]
            yield

        # ------------------------------------------------------------------
        def vstep(Vold_ap, Z, Y, P, W, tag):
            """v = Y|Y|/(Z^2+Y^2)  (== squash(Y/Z), Z>0); Vn = Vold+v."""
            zz = smp.tile([P, W], F32, tag=f"zz{tag}", name=f"zz{tag}")
            nc.gpsimd.tensor_tensor(zz[:], Z[:], Z[:], mult)
            n2 = smp.tile([P, W], F32, tag=f"n2{tag}", name=f"n2{tag}")
            nc.gpsimd.tensor_tensor(n2[:], Y[:], Y[:], mult)
            ay = smp.tile([P, W], F32, tag=f"ay{tag}", name=f"ay{tag}")
            nc.scalar.activation(ay[:], Y[:], mybir.ActivationFunctionType.Abs)
            den = smp.tile([P, W], F32, tag=f"dn{tag}", name=f"dn{tag}")
            nc.gpsimd.tensor_tensor(den[:], zz[:], n2[:], add)
            r = smp.tile([P, W], F32, tag=f"r{tag}", name=f"r{tag}")
            nc.vector.reciprocal(r[:], den[:])
            yy = smp.tile([P, W], F32, tag=f"yy{tag}", name=f"yy{tag}")
            nc.gpsimd.tensor_tensor(yy[:], Y[:], ay[:], mult)
            v = smp.tile([P, W], F32, tag=f"v{tag}", name=f"v{tag}")
            nc.gpsimd.tensor_tensor(v[:], yy[:], r[:], mult)
            Vn = smp.tile([P, W], F32, tag=f"V{tag}", name=f"V{tag}")
            nc.gpsimd.tensor_tensor(Vn[:], Vold_ap, v[:], add)
            return v, Vn

        def route_t1_iter(bg, it):
            a1t = a1t_of[bg]
            for h in range(2):
                if it == 2:
                    Vh = st["V_t1_1"][:, 8 * bg + 4 * h:8 * bg + 4 * h + 4]
                else:
                    Vh = st[("V_t1h", bg, it - 1, h)][:]
                Z = smp.tile([128, 4], F32, tag="Zt1", name="Zt1")
                Y = smp.tile([128, 4], F32, tag="Yt1", name="Yt1")
                for j in range(4):
                    bl = 4 * h + j
                    p_sl = a1t[:, bl * N:(bl + 1) * N]
                    zy_triple(p_sl, Vh[:, j:j + 1],
                              Z[:, j:j + 1], Y[:, j:j + 1])
                    if j % 2 == 1:
                        yield
                v, Vn = vstep(Vh, Z, Y, 128, 4, "t1")
                st[("V_t1h", bg, it, h)] = Vn
                if it == T:
                    nc.sync.dma_start(out1_d[bg][:, 4 * h:4 * h + 4], v[:])
                yield

        def route_a2_iter(bg, it):
            for blh in range(2):
                if it == 2:
                    Vh = st["Va2_1"][:, 2 * bg + blh:2 * bg + blh + 1]
                else:
                    Vh = st[("Va2h", bg, it - 1, blh)][:]
                Z = smp.tile([128, 1], F32, tag="Za2", name="Za2")
                Y = smp.tile([128, 1], F32, tag="Ya2", name="Ya2")
                off = (bg * 2 + blh) * N
                p_sl = a2t[:, off:off + N]
                zy_triple(p_sl, Vh[:, 0:1], Z[:, 0:1], Y[:, 0:1])
                v, Vn = vstep(Vh, Z, Y, 128, 1, "a2")
                st[("Va2h", bg, it, blh)] = Vn
                if it == T:
                    # out2[(q,ko2), (bg,blh)] device-native; host remaps
                    nc.sync.dma_start(
                        out2_d[:, 2 * bg + blh:2 * bg + blh + 1], v[:])
                yield

        # ---- emission schedule: diagonal (it2(bg) || it3(bg-1) || gen) ----
        st["m1ps"] = psM.tile([32, KO], F32, tag="m1ps", name="m1ps")

        def dbl(g):
            # two units per interleave round
            it = iter(g)
            while True:
                try:
                    next(it)
                except StopIteration:
                    return
                try:
                    next(it)
                except StopIteration:
                    pass
                yield

        _interleave(gen_phase(0))
        _interleave(m1_finalize())
        if T == 1:
            for bg in range(1, BG):
                _interleave(gen_phase(bg))
            for bg in range(BG):
                nc.sync.dma_start(out1_d[bg],
                                  st["V_t1_1"][:, 8 * bg:8 * bg + 8])
                nc.sync.dma_start(out2_d[:, 2 * bg:2 * bg + 2],
                                  st["Va2_1"][:, 2 * bg:2 * bg + 2])
        else:
            ph = 0
            while True:
                gens = []
                if ph + 1 < BG:
                    gens.append(dbl(gen_phase(ph + 1)))
                for bg in range(BG):
                    it = ph - bg + 2
                    if 2 <= it <= T:
                        gens.append(route_t1_iter(bg, it))
                        gens.append(route_a2_iter(bg, it))
                if not gens:
                    break
                _interleave(*gens)
                ph += 1

    nc.compile()
    return nc


def _get_program(T):
    if T not in _cache:
        _cache[T] = _build_program(T)
    return _cache[T]


# ----------------------------------------------------------------------------
# host-side output assembly
# ----------------------------------------------------------------------------
def _assemble(results):
    v = np.zeros((K, B, 1, 1, O), dtype=np.float32)
    for c, res in enumerate(results):
        o1 = res["out1"]          # [BG, 128=(16k+o), BL]
        o2 = res["out2"]          # [128=(q, 16(k-8)+o), 8=(bg,blh)]
        b0 = c * BSH
        # o1[bg, 16k+o, bl] -> v[k, b0+8bg+bl, 0, 0, o]
        t = o1.reshape(BG, 8, O, BL).transpose(1, 0, 3, 2)  # [k, bg, bl, o]
        v[:8, b0:b0 + BSH, 0, 0, :] = t.reshape(8, BSH, O)
        # o2[(q,16kk+o), (bg,blh)] -> v[8+kk, b0+8bg+4blh+q, 0, 0, o]
        t2 = o2.reshape(4, 2, O, BG, 2).transpose(1, 3, 4, 0, 2)
        v[8:, b0:b0 + BSH, 0, 0, :] = t2.reshape(2, BSH, O)
    return v


def run(x, routing_weights, num_iterations, trace=False):
    T = int(num_iterations)
    x = np.asarray(x, dtype=np.float32)
    w = np.asarray(routing_weights, dtype=np.float32)
    nc = _get_program(T)
    in_maps = _prep_core_inputs(x, w)
    kw = {}
    if trace:
        kw = dict(trace=True, trace_cores=list(range(NCORES)))
    res = run_bass_kernel_spmd(nc, in_maps, core_ids=list(range(NCORES)), **kw)
    return _assemble(res.results), res


def kernel(x, routing_weights, num_iterations):
    out, _ = run(x, routing_weights, num_iterations)
    return out


# revision 32
# speedup vs baseline: 1.0312x; 1.0312x over previous
"""Trainium2 Bass kernel for nn_CapsuleLayer (dynamic routing).

Math (per independent column c=(k,b,o), vector p = pred[k,b,:,o] of length N):
    logits stay proportional to p:  logits_t = p * V_t  with scalar V_t.
    iter 1: c uniform -> s1 = mean_n(p);  v1 = squash(s1); V1 = v1
    iter t: Z = sum_n exp(V*p), Y = sum_n p*exp(V*p), s = Y/Z,
            v = squash(s) = s*|s|/(1+s^2), V += v
    output = v from the last iteration.

Sharding: data-parallel over batch (32 of 256 per core, 8 cores).

Per-core device pipeline:
  pred is computed by PE as  Wr[ng].T @ xbd[bg,ng]  where xbd is a
  host-built block-diagonal slab of x (contraction = 16 n-values x 8 cin),
  so PSUM comes out column-major: rows=(k,o), free=(nl, bl).
    - "T1" rows = ko 0..127 (k0..7)  -> A1[bg] tiles [128, 8*1152] fp16
    - "A2" rows = ko 128..159 (k8,k9), bl-major: partitions (q=bl%4, ko2),
      free (bg, blh=bl//4, ng, nl) -> A2 tile [128, 9216] fp16.  Each
      route instruction then covers a full [128, 1152] slab with a
      per-partition V scale, so no j-reduce fixups are needed.
  Routing per (column-set, iteration) triple: ScalarE exp (per-partition
  scale=V, optionally fused accum_out=Z), Z otherwise via DVE
  affine_mul_reduce (out=(p*0+1)*e in-place, accum=Z), and
  Y = sum p*e via DVE affine_mul_reduce (2x mode) or GpSimd
  scalar_tensor_tensor -- statically load-balanced across engines.
"""

import sys

sys.path.insert(0, "/opt/trn_rl_repo")

from contextlib import ExitStack

import numpy as np

import concourse.bass as bass  # noqa: F401
import concourse.bacc as bacc
import concourse.tile as tile
from concourse import mybir
from concourse.bass_utils import run_bass_kernel_spmd

# ---- problem constants (hardcoded per harness contract) ----
B, N, CIN = 256, 1152, 8
K, O = 10, 16
KO = K * O            # 160
NCORES = 8
BSH = B // NCORES     # 32 batch per core
BG, BL = 4, 8         # batch groups x lanes (BSH = BG*BL)
NG, NL = 72, 16       # n-groups x n-lanes (N = NG*NL)
F32 = mybir.dt.float32
F16 = mybir.dt.float16

# ---- engine load-balance knobs ----
ZD_MOD = 0            # Z on DVE amr when idx % ZD_MOD == 0 (0 = always Act)
EVAC_ACT_MOD = (1, 3, 5, 5)   # per-bg: evac on Act every Nth (0 = never)

_cache = {}


# ----------------------------------------------------------------------------
# host-side input prep
# ----------------------------------------------------------------------------
def _prep_shared(w):
    # Wr[ng, 8*nl+i, 16*k+o] = w[k, 16*ng+nl, i, o]; ship partition-major
    wr = np.transpose(
        w.reshape(K, NG, NL, CIN, O), (1, 2, 3, 0, 4)
    ).reshape(NG, 128, KO).astype(np.float16)
    wr = np.ascontiguousarray(np.transpose(wr, (1, 0, 2)).reshape(128, NG * KO))
    ident32 = np.eye(32, dtype=np.float32)
    return wr, ident32


def _prep_core_inputs(x, w):
    wr, ident32 = _prep_shared(w)
    in_maps = []
    for c in range(NCORES):
        xc = x[c * BSH:(c + 1) * BSH]                          # [32, N, CIN]
        # xs[ng, 8*nl+i, b] = xc[b, 16*ng+nl, i]
        xs = np.transpose(
            xc.reshape(BSH, NG, NL, CIN), (1, 2, 3, 0)
        ).reshape(NG, 128, BSH).astype(np.float16)
        xs = np.ascontiguousarray(
            np.transpose(xs, (1, 0, 2)).reshape(128, NG * BSH))
        # xbd[bg, ng, (nl',i), (nl,bl)] = xc[8bg+bl, 16ng+nl, i] * (nl==nl')
        xbd = np.zeros((BG, NG, NL, CIN, NL, BL), dtype=np.float16)
        xs5 = np.transpose(
            xc.reshape(BG, BL, NG, NL, CIN), (0, 2, 3, 4, 1)
        ).astype(np.float16)                                   # [bg,ng,nl,i,bl]
        for r in range(NL):
            xbd[:, :, r, :, r, :] = xs5[:, :, r, :, :]
        xbd = np.ascontiguousarray(
            np.transpose(xbd.reshape(BG, NG, 128, 128),
                         (0, 2, 1, 3)).reshape(BG, 128, NG * 128))
        in_maps.append({
            "xbd": xbd, "xs": xs, "wr": wr, "ident32": ident32,
        })
    return in_maps


# ----------------------------------------------------------------------------
# device program
# ----------------------------------------------------------------------------
def _interleave(*gens):
    gens = list(gens)
    while gens:
        nxt = []
        for g in gens:
            try:
                next(g)
                nxt.append(g)
            except StopIteration:
                pass
        gens = nxt


def _build_program(T):
    nc = bacc.Bacc("TRN2", target_bir_lowering=False, debug=False,
                   enable_asserts=False)

    xbd_d = nc.dram_tensor("xbd", [BG, 128, NG * 128], F16, kind="ExternalInput").ap()
    xs_d = nc.dram_tensor("xs", [128, NG * BSH], F16, kind="ExternalInput").ap()
    wr_d = nc.dram_tensor("wr", [128, NG * KO], F16, kind="ExternalInput").ap()
    id_d = nc.dram_tensor("ident32", [32, 32], F32, kind="ExternalInput").ap()
    out1_d = nc.dram_tensor("out1", [BG, 128, BL], F32, kind="ExternalOutput").ap()
    out2_d = nc.dram_tensor("out2", [128, 8], F32, kind="ExternalOutput").ap()

    mult = mybir.AluOpType.mult
    add = mybir.AluOpType.add
    EXP = mybir.ActivationFunctionType.Exp
    smp_bufs = max(14, 8 * (T - 1) + 2)

    with tile.TileContext(nc) as tc, ExitStack() as ctx:
        consts = ctx.enter_context(tc.tile_pool(name="consts", bufs=1))
        a1p = ctx.enter_context(tc.tile_pool(name="a1", bufs=3))
        a2p = ctx.enter_context(tc.tile_pool(name="a2", bufs=1))
        xbdp = ctx.enter_context(tc.tile_pool(name="xbd", bufs=2))
        ep = ctx.enter_context(tc.tile_pool(name="e", bufs=8))
        smp = ctx.enter_context(tc.tile_pool(name="sm", bufs=smp_bufs))
        psA = ctx.enter_context(tc.tile_pool(name="psA", bufs=2, space="PSUM"))
        psB = ctx.enter_context(tc.tile_pool(name="psB", bufs=2, space="PSUM"))
        psM = ctx.enter_context(tc.tile_pool(name="psM", bufs=1, space="PSUM"))
        psT = ctx.enter_context(tc.tile_pool(name="psT", bufs=1, space="PSUM"))

        # ---- resident inputs ----
        # bg0's xbd slab is the first matmul dependency: interleave its
        # chunks ahead of/between the weight chunks in DMA emission order.
        wrs = consts.tile([128, NG * KO], F16, tag="wrs", name="wrs")
        xsal = consts.tile([128, NG * BSH], F16, tag="xsal", name="xsal")
        xbt0 = xbdp.tile([128, NG * 128], F16, tag="xbd", name="xbd")
        WCH = 4
        for ch in range(WCH):
            if ch < 3:
                c0, c1 = ch * NG // 3, (ch + 1) * NG // 3
                nc.sync.dma_start(xbt0[:, c0 * 128:c1 * 128],
                                  xbd_d[0, :, c0 * 128:c1 * 128])
            g0, g1 = ch * NG // WCH, (ch + 1) * NG // WCH
            nc.sync.dma_start(wrs[:, g0 * KO:g1 * KO], wr_d[:, g0 * KO:g1 * KO])
            nc.sync.dma_start(xsal[:, g0 * BSH:g1 * BSH],
                              xs_d[:, g0 * BSH:g1 * BSH])
        id32 = consts.tile([32, 32], F32, tag="id32", name="id32")
        nc.sync.dma_start(id32[:], id_d)
        ones1 = consts.tile([128, 1], F32, tag="ones1", name="ones1")
        nc.vector.memset(ones1[:], 1.0)

        # A2 accumulator: rows (q=bl%4, ko2); free (bg, blh=bl//4, ng, nl)
        a2t = a2p.tile([128, BG * 2 * NG * NL], F16, tag="a2", name="a2")
        a2v = a2t[:].rearrange("p (G h g l) -> p G h g l",
                               G=BG, h=2, g=NG, l=NL)

        st = {}
        a1t_of = {}
        evac_tgl = [0]
        tri = [0]

        def evac_copy(dst, src, bg=0):
            mod = EVAC_ACT_MOD[bg]
            if mod and evac_tgl[0] % mod == 0:
                nc.scalar.copy(dst, src)
            else:
                nc.vector.tensor_copy(dst, src)
            evac_tgl[0] += 1

        def zy_triple(p_sl, Vscale, Z_ap, Y_ap):
            """exp + Z + Y for one [128, N] column-set; engines per knobs."""
            i = tri[0]
            tri[0] += 1
            e = ep.tile([128, N], F16, tag="e", name="e")
            z_act = (ZD_MOD == 0) or (i % ZD_MOD) != 0
            if z_act:
                nc.scalar.activation(e[:], p_sl, EXP, scale=Vscale,
                                     accum_out=Z_ap)
            else:
                nc.scalar.activation(e[:], p_sl, EXP, scale=Vscale)
                nc.vector.affine_mul_reduce(
                    out=e[:], accum_out=Z_ap, in0=p_sl, in1=e[:],
                    scale=0.0, bias=1.0)
            nc.vector.affine_mul_reduce(
                out=e[:], accum_out=Y_ap, in0=e[:], in1=p_sl,
                scale=1.0, bias=0.0)

        def squash(s_ap, P, W):
            """v = s*|s|/(1+s*s) as a fresh [P, W] f32 tile"""
            n2 = smp.tile([P, W], F32, tag=f"sq_n2_{P}_{W}", name=f"sq_n2_{P}_{W}")
            nc.vector.tensor_tensor(n2[:], s_ap, s_ap, mult)
            d = smp.tile([P, W], F32, tag=f"sq_d_{P}_{W}", name=f"sq_d_{P}_{W}")
            nc.vector.tensor_scalar_add(d[:], n2[:], 1.0)
            r = smp.tile([P, W], F32, tag=f"sq_r_{P}_{W}", name=f"sq_r_{P}_{W}")
            nc.vector.reciprocal(r[:], d[:])
            a = smp.tile([P, W], F32, tag=f"sq_a_{P}_{W}", name=f"sq_a_{P}_{W}")
            nc.scalar.activation(a[:], s_ap, mybir.ActivationFunctionType.Abs)
            t = smp.tile([P, W], F32, tag=f"sq_t_{P}_{W}", name=f"sq_t_{P}_{W}")
            nc.vector.tensor_tensor(t[:], s_ap, a[:], mult)
            v = smp.tile([P, W], F32, tag=f"sq_v_{P}_{W}", name=f"sq_v_{P}_{W}")
            nc.vector.tensor_tensor(v[:], t[:], r[:], mult)
            return v

        # ------------------------------------------------------------------
        def gen_phase(bg):
            a1t = a1p.tile([128, BL * N], F16, tag="a1", name="a1")
            st[("a1", bg)] = a1t
            a1t_of[bg] = a1t
            a1v = a1t[:].rearrange("p (b g l) -> p g l b", b=BL, g=NG, l=NL)
            if bg == 0:
                xbt = xbt0
            else:
                xbt = xbdp.tile([128, NG * 128], F16, tag="xbd", name="xbd")
                for ch in range(3):
                    c0, c1 = ch * NG // 3, (ch + 1) * NG // 3
                    nc.sync.dma_start(xbt[:, c0 * 128:c1 * 128],
                                      xbd_d[bg, :, c0 * 128:c1 * 128])
            # xbd columns (nl, bl) split as (l, h=bl//4, q=bl%4)
            xbq = xbt[:].rearrange("p (g l h q) -> p g q l h",
                                   g=NG, l=NL, h=2, q=4)
            pa = None
            pb = None
            pa_q0 = 0
            pb_q0 = 0
            NQ = NG // 4
            for Qn in range(NQ):              # 18 blocks of 4 ng
                if Qn % 2 == 0:
                    pa = psA.tile([128, 1024], F32, tag="psA", name="psA")
                    pa_q0 = Qn
                    pb = psB.tile([128, 256], F32, tag="psB", name="psB")
                    pb_q0 = Qn
                for j in range(4):
                    ng = 4 * Qn + j
                    w0 = wrs[:, ng * KO:ng * KO + 128]
                    w1 = wrs[:, ng * KO + 128:ng * KO + KO]
                    rhs = xbt[:, ng * 128:(ng + 1) * 128]
                    jj = (Qn - pa_q0) * 4 + j
                    nc.tensor.matmul(pa[:, jj * 128:(jj + 1) * 128],
                                     w0, rhs, start=True, stop=True)
                    ngi = (Qn - pb_q0) * 4 + j
                    for q in range(4):
                        nc.tensor.matmul(
                            pb[32 * q:32 * q + 32, ngi * 32:(ngi + 1) * 32],
                            w1, xbq[:, ng, q], start=True, stop=True,
                            tile_position=(0, 32 * q))
                    if bg == 0:
                        nc.tensor.matmul(
                            st["m1ps"][:],
                            xsal[:, ng * BSH:(ng + 1) * BSH],
                            wrs[:, ng * KO:(ng + 1) * KO],
                            start=(ng == 0), stop=(ng == NG - 1))
                # evacuate psA -> A1[bg]; dst/src iteration order = (g, l, b)
                if Qn % 2 == 1:
                    evac_copy(a1v[:, 4 * pa_q0:4 * pa_q0 + 8, :, :],
                              pa[:].rearrange("p (g l b) -> p g l b",
                                              g=8, l=NL, b=BL), bg=bg)
                    ng0 = 4 * pb_q0
                    evac_copy(
                        a2v[:, bg, :, ng0:ng0 + 8, :],
                        pb[:].rearrange("p (g l h) -> p h g l",
                                        g=8, l=NL, h=2), bg=bg)
                yield

        # ------------------------------------------------------------------
        def m1_finalize():
            m1s = smp.tile([32, KO], F32, tag="m1s", name="m1s")
            nc.vector.tensor_copy(m1s[:], st["m1ps"][:])
            t1 = psT.tile([128, 32], F32, tag="psT", name="psT")
            nc.tensor.transpose(t1[:], m1s[:, 0:128], id32[:])
            s1 = smp.tile([128, 32], F32, tag="s1t1", name="s1t1")
            nc.vector.tensor_scalar_mul(s1[:], t1[:], 1.0 / N)
            v1 = squash(s1[:], 128, 32)
            st["V_t1_1"] = v1              # [128=(ko), 32=(bg,bl)]
            t2 = psT.tile([128, 32], F32, tag="psT", name="psT")
            nc.tensor.transpose(t2[:32, :], m1s[:, 128:KO], id32[:])
            s1a = smp.tile([32, 32], F32, tag="s1a2", name="s1a2")
            nc.vector.tensor_scalar_mul(s1a[:], t2[:32, :], 1.0 / N)
            v1a = squash(s1a[:], 32, 32)   # [32=(ko2), 32=(bg,bl)]
            yield
            # bl-major replication: Vb[(q,ko2), (bg,blh)] = v1a[ko2, 8bg+4blh+q]
            v1aq = v1a[:].rearrange("k (g h q) -> k q g h", g=BG, h=2, q=4)
            vb = psT.tile([128, 32], F32, tag="psT", name="psT")
            for q in range(4):
                nc.tensor.matmul(vb[32 * q:32 * q + 32, 0:8], id32[:],
                                 v1aq[:, q], start=True, stop=True,
                                 tile_position=(0, 32 * q))
            vbig = smp.tile([128, 8], F32, tag="va2bm", name="va2bm")
            nc.vector.tensor_copy(vbig[:], vb[:, 0:8])
            st["Va2_1"] = vbig             # [128=(q,ko2), 8=(bg,blh)]
            yield

        # ------------------------------------------------------------------
        def vstep(Vold_ap, Z, Y, P, W, tag):
            """v = Y|Y|/(Z^2+Y^2)  (== squash(Y/Z), Z>0); Vn = Vold+v."""
            zz = smp.tile([P, W], F32, tag=f"zz{tag}", name=f"zz{tag}")
            nc.gpsimd.tensor_tensor(zz[:], Z[:], Z[:], mult)
            n2 = smp.tile([P, W], F32, tag=f"n2{tag}", name=f"n2{tag}")
            nc.gpsimd.tensor_tensor(n2[:], Y[:], Y[:], mult)
            ay = smp.tile([P, W], F32, tag=f"ay{tag}", name=f"ay{tag}")
            nc.scalar.activation(ay[:], Y[:], mybir.ActivationFunctionType.Abs)
            den = smp.tile([P, W], F32, tag=f"dn{tag}", name=f"dn{tag}")
            nc.gpsimd.tensor_tensor(den[:], zz[:], n2[:], add)
            r = smp.tile([P, W], F32, tag=f"r{tag}", name=f"r{tag}")
            nc.vector.reciprocal(r[:], den[:])
            yy = smp.tile([P, W], F32, tag=f"yy{tag}", name=f"yy{tag}")
            nc.gpsimd.tensor_tensor(yy[:], Y[:], ay[:], mult)
            v = smp.tile([P, W], F32, tag=f"v{tag}", name=f"v{tag}")
            nc.gpsimd.tensor_tensor(v[:], yy[:], r[:], mult)
            Vn = smp.tile([P, W], F32, tag=f"V{tag}", name=f"V{tag}")
            nc.gpsimd.tensor_tensor(Vn[:], Vold_ap, v[:], add)
            return v, Vn

        def route_t1_iter(bg, it):
            a1t = a1t_of[bg]
            for h in range(2):
                if it == 2:
                    Vh = st["V_t1_1"][:, 8 * bg + 4 * h:8 * bg + 4 * h + 4]
                else:
                    Vh = st[("V_t1h", bg, it - 1, h)][:]
                Z = smp.tile([128, 4], F32, tag="Zt1", name="Zt1")
                Y = smp.tile([128, 4], F32, tag="Yt1", name="Yt1")
                for j in range(4):
                    bl = 4 * h + j
                    p_sl = a1t[:, bl * N:(bl + 1) * N]
                    zy_triple(p_sl, Vh[:, j:j + 1],
                              Z[:, j:j + 1], Y[:, j:j + 1])
                    if j % 2 == 1:
                        yield
                v, Vn = vstep(Vh, Z, Y, 128, 4, "t1")
                st[("V_t1h", bg, it, h)] = Vn
                if it == T:
                    nc.sync.dma_start(out1_d[bg][:, 4 * h:4 * h + 4], v[:])
                yield

        def route_a2_iter(bg, it):
            for blh in range(2):
                if it == 2:
                    Vh = st["Va2_1"][:, 2 * bg + blh:2 * bg + blh + 1]
                else:
                    Vh = st[("Va2h", bg, it - 1, blh)][:]
                Z = smp.tile([128, 1], F32, tag="Za2", name="Za2")
                Y = smp.tile([128, 1], F32, tag="Ya2", name="Ya2")
                off = (bg * 2 + blh) * N
                p_sl = a2t[:, off:off + N]
                zy_triple(p_sl, Vh[:, 0:1], Z[:, 0:1], Y[:, 0:1])
                v, Vn = vstep(Vh, Z, Y, 128, 1, "a2")
                st[("Va2h", bg, it, blh)] = Vn
                if it == T:
                    # out2[(q,ko2), (bg,blh)] device-native; host remaps
                    nc.sync.dma_start(
                        out2_d[:, 2 * bg + blh:2 * bg + blh + 1], v[:])
                yield

        # ---- emission schedule: diagonal (it2(bg) || it3(bg-1) || gen) ----
        st["m1ps"] = psM.tile([32, KO], F32, tag="m1ps", name="m1ps")

        def dbl(g):
            # two units per interleave round
            it = iter(g)
            while True:
                try:
                    next(it)
                except StopIteration:
                    return
                try:
                    next(it)
                except StopIteration:
                    pass
                yield

        _interleave(gen_phase(0))
        _interleave(m1_finalize())
        if T == 1:
            for bg in range(1, BG):
                _interleave(gen_phase(bg))
            for bg in range(BG):
                nc.sync.dma_start(out1_d[bg],
                                  st["V_t1_1"][:, 8 * bg:8 * bg + 8])
                nc.sync.dma_start(out2_d[:, 2 * bg:2 * bg + 2],
                                  st["Va2_1"][:, 2 * bg:2 * bg + 2])
        else:
            ph = 0
            while True:
                gens = []
                if ph + 1 < BG:
                    gens.append(dbl(gen_phase(ph + 1)))
                for bg in range(BG):
                    it = ph - bg + 2
                    if 2 <= it <= T:
                        gens.append(route_t1_iter(bg, it))
                        gens.append(route_a2_iter(bg, it))
                if not gens:
                    break
                _interleave(*gens)
                ph += 1

    nc.compile()
    return nc


def _get_program(T):
    if T not in _cache:
        _cache[T] = _build_program(T)
    return _cache[T]


# ----------------------------------------------------------------------------
# host-side output assembly
# ----------------------------------------------------------------------------
def _assemble(results):
    v = np.zeros((K, B, 1, 1, O), dtype=np.float32)
    for c, res in enumerate(results):
        o1 = res["out1"]          # [BG, 128=(16k+o), BL]
        o2 = res["out2"]          # [128=(q, 16(k-8)+o), 8=(bg,blh)]
        b0 = c * BSH
        # o1[bg, 16k+o, bl] -> v[k, b0+8bg+bl, 0, 0, o]
        t = o1.reshape(BG, 8, O, BL).transpose(1, 0, 3, 2)  # [k, bg, bl, o]
        v[:8, b0:b0 + BSH, 0, 0, :] = t.reshape(8, BSH, O)
        # o2[(q,16kk+o), (bg,blh)] -> v[8+kk, b0+8bg+4blh+q, 0, 0, o]
        t2 = o2.reshape(4, 2, O, BG, 2).transpose(1, 3, 4, 0, 2)
        v[8:, b0:b0 + BSH, 0, 0, :] = t2.reshape(2, BSH, O)
    return v


def run(x, routing_weights, num_iterations, trace=False):
    T = int(num_iterations)
    x = np.asarray(x, dtype=np.float32)
    w = np.asarray(routing_weights, dtype=np.float32)
    nc = _get_program(T)
    in_maps = _prep_core_inputs(x, w)
    kw = {}
    if trace:
        kw = dict(trace=True, trace_cores=list(range(NCORES)))
    res = run_bass_kernel_spmd(nc, in_maps, core_ids=list(range(NCORES)), **kw)
    return _assemble(res.results), res


def kernel(x, routing_weights, num_iterations):
    out, _ = run(x, routing_weights, num_iterations)
    return out


# revision 33
# speedup vs baseline: 1.0617x; 1.0295x over previous
"""Trainium2 Bass kernel for nn_CapsuleLayer (dynamic routing).

Math (per independent column c=(k,b,o), vector p = pred[k,b,:,o] of length N):
    logits stay proportional to p:  logits_t = p * V_t  with scalar V_t.
    iter 1: c uniform -> s1 = mean_n(p);  v1 = squash(s1); V1 = v1
    iter t: Z = sum_n exp(V*p), Y = sum_n p*exp(V*p), s = Y/Z,
            v = squash(s) = s*|s|/(1+s^2), V += v
    output = v from the last iteration.

Sharding: data-parallel over batch (32 of 256 per core, 8 cores).

Per-core device pipeline:
  pred is computed by PE as  Wr[ng].T @ xbd[bg,ng]  where xbd is a
  host-built block-diagonal slab of x (contraction = 16 n-values x 8 cin),
  so PSUM comes out column-major: rows=(k,o), free=(nl, bl).
    - "T1" rows = ko 0..127 (k0..7)  -> A1[bg] tiles [128, 8*1152] fp16
    - "A2" rows = ko 128..159 (k8,k9), bl-major: partitions (q=bl%4, ko2),
      free (bg, blh=bl//4, ng, nl) -> A2 tile [128, 9216] fp16.  Each
      route instruction then covers a full [128, 1152] slab with a
      per-partition V scale, so no j-reduce fixups are needed.
  Routing per (column-set, iteration) triple: ScalarE exp (per-partition
  scale=V, optionally fused accum_out=Z), Z otherwise via DVE
  affine_mul_reduce (out=(p*0+1)*e in-place, accum=Z), and
  Y = sum p*e via DVE affine_mul_reduce (2x mode) or GpSimd
  scalar_tensor_tensor -- statically load-balanced across engines.
"""

import sys

sys.path.insert(0, "/opt/trn_rl_repo")

from contextlib import ExitStack

import numpy as np

import concourse.bass as bass  # noqa: F401
import concourse.bacc as bacc
import concourse.tile as tile
from concourse import mybir
from concourse.bass_utils import run_bass_kernel_spmd

# ---- problem constants (hardcoded per harness contract) ----
B, N, CIN = 256, 1152, 8
K, O = 10, 16
KO = K * O            # 160
NCORES = 8
BSH = B // NCORES     # 32 batch per core
BG, BL = 4, 8         # batch groups x lanes (BSH = BG*BL)
NG, NL = 72, 16       # n-groups x n-lanes (N = NG*NL)
F32 = mybir.dt.float32
F16 = mybir.dt.float16

# ---- engine load-balance knobs ----
ZD_MOD = 0            # Z on DVE amr when idx % ZD_MOD == 0 (0 = always Act)
EVAC_ACT_MOD = (1, 3, 4, 4)   # per-bg: evac on Act every Nth (0 = never)

_cache = {}


# ----------------------------------------------------------------------------
# host-side input prep
# ----------------------------------------------------------------------------
def _prep_shared(w):
    # Wr[ng, 8*nl+i, 16*k+o] = w[k, 16*ng+nl, i, o]; ship partition-major
    wr = np.transpose(
        w.reshape(K, NG, NL, CIN, O), (1, 2, 3, 0, 4)
    ).reshape(NG, 128, KO).astype(np.float16)
    wr = np.ascontiguousarray(np.transpose(wr, (1, 0, 2)).reshape(128, NG * KO))
    ident32 = np.eye(32, dtype=np.float32)
    return wr, ident32


def _prep_core_inputs(x, w):
    wr, ident32 = _prep_shared(w)
    in_maps = []
    for c in range(NCORES):
        xc = x[c * BSH:(c + 1) * BSH]                          # [32, N, CIN]
        # xs[ng, 8*nl+i, b] = xc[b, 16*ng+nl, i]
        xs = np.transpose(
            xc.reshape(BSH, NG, NL, CIN), (1, 2, 3, 0)
        ).reshape(NG, 128, BSH).astype(np.float16)
        xs = np.ascontiguousarray(
            np.transpose(xs, (1, 0, 2)).reshape(128, NG * BSH))
        # xbd[bg, ng, (nl',i), (nl,bl)] = xc[8bg+bl, 16ng+nl, i] * (nl==nl')
        xbd = np.zeros((BG, NG, NL, CIN, NL, BL), dtype=np.float16)
        xs5 = np.transpose(
            xc.reshape(BG, BL, NG, NL, CIN), (0, 2, 3, 4, 1)
        ).astype(np.float16)                                   # [bg,ng,nl,i,bl]
        for r in range(NL):
            xbd[:, :, r, :, r, :] = xs5[:, :, r, :, :]
        xbd = np.ascontiguousarray(
            np.transpose(xbd.reshape(BG, NG, 128, 128),
                         (0, 2, 1, 3)).reshape(BG, 128, NG * 128))
        in_maps.append({
            "xbd": xbd, "xs": xs, "wr": wr, "ident32": ident32,
        })
    return in_maps


# ----------------------------------------------------------------------------
# device program
# ----------------------------------------------------------------------------
def _interleave(*gens):
    gens = list(gens)
    while gens:
        nxt = []
        for g in gens:
            try:
                next(g)
                nxt.append(g)
            except StopIteration:
                pass
        gens = nxt


def _build_program(T):
    nc = bacc.Bacc("TRN2", target_bir_lowering=False, debug=False,
                   enable_asserts=False)

    xbd_d = nc.dram_tensor("xbd", [BG, 128, NG * 128], F16, kind="ExternalInput").ap()
    xs_d = nc.dram_tensor("xs", [128, NG * BSH], F16, kind="ExternalInput").ap()
    wr_d = nc.dram_tensor("wr", [128, NG * KO], F16, kind="ExternalInput").ap()
    id_d = nc.dram_tensor("ident32", [32, 32], F32, kind="ExternalInput").ap()
    out1_d = nc.dram_tensor("out1", [BG, 128, BL], F32, kind="ExternalOutput").ap()
    out2_d = nc.dram_tensor("out2", [128, 8], F32, kind="ExternalOutput").ap()

    mult = mybir.AluOpType.mult
    add = mybir.AluOpType.add
    EXP = mybir.ActivationFunctionType.Exp
    smp_bufs = max(14, 8 * (T - 1) + 2)

    with tile.TileContext(nc) as tc, ExitStack() as ctx:
        consts = ctx.enter_context(tc.tile_pool(name="consts", bufs=1))
        a1p = ctx.enter_context(tc.tile_pool(name="a1", bufs=3))
        a2p = ctx.enter_context(tc.tile_pool(name="a2", bufs=1))
        xbdp = ctx.enter_context(tc.tile_pool(name="xbd", bufs=2))
        ep = ctx.enter_context(tc.tile_pool(name="e", bufs=8))
        smp = ctx.enter_context(tc.tile_pool(name="sm", bufs=smp_bufs))
        psA = ctx.enter_context(tc.tile_pool(name="psA", bufs=2, space="PSUM"))
        psB = ctx.enter_context(tc.tile_pool(name="psB", bufs=2, space="PSUM"))
        psM = ctx.enter_context(tc.tile_pool(name="psM", bufs=1, space="PSUM"))
        psT = ctx.enter_context(tc.tile_pool(name="psT", bufs=1, space="PSUM"))

        # ---- resident inputs ----
        # bg0's xbd slab is the first matmul dependency: interleave its
        # chunks ahead of/between the weight chunks in DMA emission order.
        wrs = consts.tile([128, NG * KO], F16, tag="wrs", name="wrs")
        xsal = consts.tile([128, NG * BSH], F16, tag="xsal", name="xsal")
        xbt0 = xbdp.tile([128, NG * 128], F16, tag="xbd", name="xbd")
        WCH = 4
        for ch in range(WCH):
            if ch < 3:
                c0, c1 = ch * NG // 3, (ch + 1) * NG // 3
                nc.sync.dma_start(xbt0[:, c0 * 128:c1 * 128],
                                  xbd_d[0, :, c0 * 128:c1 * 128])
            g0, g1 = ch * NG // WCH, (ch + 1) * NG // WCH
            nc.sync.dma_start(wrs[:, g0 * KO:g1 * KO], wr_d[:, g0 * KO:g1 * KO])
            nc.sync.dma_start(xsal[:, g0 * BSH:g1 * BSH],
                              xs_d[:, g0 * BSH:g1 * BSH])
        id32 = consts.tile([32, 32], F32, tag="id32", name="id32")
        nc.sync.dma_start(id32[:], id_d)
        ones1 = consts.tile([128, 1], F32, tag="ones1", name="ones1")
        nc.vector.memset(ones1[:], 1.0)

        # A2 accumulator: rows (q=bl%4, ko2); free (bg, blh=bl//4, ng, nl)
        a2t = a2p.tile([128, BG * 2 * NG * NL], F16, tag="a2", name="a2")
        a2v = a2t[:].rearrange("p (G h g l) -> p G h g l",
                               G=BG, h=2, g=NG, l=NL)

        st = {}
        a1t_of = {}
        evac_tgl = [0]
        tri = [0]

        def evac_copy(dst, src, bg=0):
            mod = EVAC_ACT_MOD[bg]
            if mod and evac_tgl[0] % mod == 0:
                nc.scalar.copy(dst, src)
            else:
                nc.vector.tensor_copy(dst, src)
            evac_tgl[0] += 1

        def zy_triple(p_sl, Vscale, Z_ap, Y_ap):
            """exp + Z + Y for one [128, N] column-set; engines per knobs."""
            i = tri[0]
            tri[0] += 1
            e = ep.tile([128, N], F16, tag="e", name="e")
            z_act = (ZD_MOD == 0) or (i % ZD_MOD) != 0
            if z_act:
                nc.scalar.activation(e[:], p_sl, EXP, scale=Vscale,
                                     accum_out=Z_ap)
            else:
                nc.scalar.activation(e[:], p_sl, EXP, scale=Vscale)
                nc.vector.affine_mul_reduce(
                    out=e[:], accum_out=Z_ap, in0=p_sl, in1=e[:],
                    scale=0.0, bias=1.0)
            nc.vector.affine_mul_reduce(
                out=e[:], accum_out=Y_ap, in0=e[:], in1=p_sl,
                scale=1.0, bias=0.0)

        def squash(s_ap, P, W):
            """v = s*|s|/(1+s*s) as a fresh [P, W] f32 tile"""
            n2 = smp.tile([P, W], F32, tag=f"sq_n2_{P}_{W}", name=f"sq_n2_{P}_{W}")
            nc.vector.tensor_tensor(n2[:], s_ap, s_ap, mult)
            d = smp.tile([P, W], F32, tag=f"sq_d_{P}_{W}", name=f"sq_d_{P}_{W}")
            nc.vector.tensor_scalar_add(d[:], n2[:], 1.0)
            r = smp.tile([P, W], F32, tag=f"sq_r_{P}_{W}", name=f"sq_r_{P}_{W}")
            nc.vector.reciprocal(r[:], d[:])
            a = smp.tile([P, W], F32, tag=f"sq_a_{P}_{W}", name=f"sq_a_{P}_{W}")
            nc.scalar.activation(a[:], s_ap, mybir.ActivationFunctionType.Abs)
            t = smp.tile([P, W], F32, tag=f"sq_t_{P}_{W}", name=f"sq_t_{P}_{W}")
            nc.vector.tensor_tensor(t[:], s_ap, a[:], mult)
            v = smp.tile([P, W], F32, tag=f"sq_v_{P}_{W}", name=f"sq_v_{P}_{W}")
            nc.vector.tensor_tensor(v[:], t[:], r[:], mult)
            return v

        # ------------------------------------------------------------------
        def gen_phase(bg):
            a1t = a1p.tile([128, BL * N], F16, tag="a1", name="a1")
            st[("a1", bg)] = a1t
            a1t_of[bg] = a1t
            a1v = a1t[:].rearrange("p (b g l) -> p g l b", b=BL, g=NG, l=NL)
            if bg == 0:
                xbt = xbt0
            else:
                xbt = xbdp.tile([128, NG * 128], F16, tag="xbd", name="xbd")
                for ch in range(3):
                    c0, c1 = ch * NG // 3, (ch + 1) * NG // 3
                    nc.sync.dma_start(xbt[:, c0 * 128:c1 * 128],
                                      xbd_d[bg, :, c0 * 128:c1 * 128])
            # xbd columns (nl, bl) split as (l, h=bl//4, q=bl%4)
            xbq = xbt[:].rearrange("p (g l h q) -> p g q l h",
                                   g=NG, l=NL, h=2, q=4)
            pa = None
            pb = None
            pa_q0 = 0
            pb_q0 = 0
            NQ = NG // 4
            for Qn in range(NQ):              # 18 blocks of 4 ng
                if Qn % 2 == 0:
                    pa = psA.tile([128, 1024], F32, tag="psA", name="psA")
                    pa_q0 = Qn
                    pb = psB.tile([128, 256], F32, tag="psB", name="psB")
                    pb_q0 = Qn
                for j in range(4):
                    ng = 4 * Qn + j
                    w0 = wrs[:, ng * KO:ng * KO + 128]
                    w1 = wrs[:, ng * KO + 128:ng * KO + KO]
                    rhs = xbt[:, ng * 128:(ng + 1) * 128]
                    jj = (Qn - pa_q0) * 4 + j
                    nc.tensor.matmul(pa[:, jj * 128:(jj + 1) * 128],
                                     w0, rhs, start=True, stop=True)
                    ngi = (Qn - pb_q0) * 4 + j
                    for q in range(4):
                        nc.tensor.matmul(
                            pb[32 * q:32 * q + 32, ngi * 32:(ngi + 1) * 32],
                            w1, xbq[:, ng, q], start=True, stop=True,
                            tile_position=(0, 32 * q))
                    if bg == 0:
                        nc.tensor.matmul(
                            st["m1ps"][:],
                            xsal[:, ng * BSH:(ng + 1) * BSH],
                            wrs[:, ng * KO:(ng + 1) * KO],
                            start=(ng == 0), stop=(ng == NG - 1))
                # evacuate psA -> A1[bg]; dst/src iteration order = (g, l, b)
                if Qn % 2 == 1:
                    evac_copy(a1v[:, 4 * pa_q0:4 * pa_q0 + 8, :, :],
                              pa[:].rearrange("p (g l b) -> p g l b",
                                              g=8, l=NL, b=BL), bg=bg)
                    ng0 = 4 * pb_q0
                    evac_copy(
                        a2v[:, bg, :, ng0:ng0 + 8, :],
                        pb[:].rearrange("p (g l h) -> p h g l",
                                        g=8, l=NL, h=2), bg=bg)
                yield

        # ------------------------------------------------------------------
        def m1_finalize():
            m1s = smp.tile([32, KO], F32, tag="m1s", name="m1s")
            nc.vector.tensor_copy(m1s[:], st["m1ps"][:])
            t1 = psT.tile([128, 32], F32, tag="psT", name="psT")
            nc.tensor.transpose(t1[:], m1s[:, 0:128], id32[:])
            s1 = smp.tile([128, 32], F32, tag="s1t1", name="s1t1")
            nc.vector.tensor_scalar_mul(s1[:], t1[:], 1.0 / N)
            v1 = squash(s1[:], 128, 32)
            st["V_t1_1"] = v1              # [128=(ko), 32=(bg,bl)]
            t2 = psT.tile([128, 32], F32, tag="psT", name="psT")
            nc.tensor.transpose(t2[:32, :], m1s[:, 128:KO], id32[:])
            s1a = smp.tile([32, 32], F32, tag="s1a2", name="s1a2")
            nc.vector.tensor_scalar_mul(s1a[:], t2[:32, :], 1.0 / N)
            v1a = squash(s1a[:], 32, 32)   # [32=(ko2), 32=(bg,bl)]
            yield
            # bl-major replication: Vb[(q,ko2), (bg,blh)] = v1a[ko2, 8bg+4blh+q]
            v1aq = v1a[:].rearrange("k (g h q) -> k q g h", g=BG, h=2, q=4)
            vb = psT.tile([128, 32], F32, tag="psT", name="psT")
            for q in range(4):
                nc.tensor.matmul(vb[32 * q:32 * q + 32, 0:8], id32[:],
                                 v1aq[:, q], start=True, stop=True,
                                 tile_position=(0, 32 * q))
            vbig = smp.tile([128, 8], F32, tag="va2bm", name="va2bm")
            nc.vector.tensor_copy(vbig[:], vb[:, 0:8])
            st["Va2_1"] = vbig             # [128=(q,ko2), 8=(bg,blh)]
            yield

        # ------------------------------------------------------------------
        def vstep(Vold_ap, Z, Y, P, W, tag):
            """v = Y|Y|/(Z^2+Y^2)  (== squash(Y/Z), Z>0); Vn = Vold+v."""
            zz = smp.tile([P, W], F32, tag=f"zz{tag}", name=f"zz{tag}")
            nc.gpsimd.tensor_tensor(zz[:], Z[:], Z[:], mult)
            n2 = smp.tile([P, W], F32, tag=f"n2{tag}", name=f"n2{tag}")
            nc.gpsimd.tensor_tensor(n2[:], Y[:], Y[:], mult)
            ay = smp.tile([P, W], F32, tag=f"ay{tag}", name=f"ay{tag}")
            nc.scalar.activation(ay[:], Y[:], mybir.ActivationFunctionType.Abs)
            den = smp.tile([P, W], F32, tag=f"dn{tag}", name=f"dn{tag}")
            nc.gpsimd.tensor_tensor(den[:], zz[:], n2[:], add)
            r = smp.tile([P, W], F32, tag=f"r{tag}", name=f"r{tag}")
            nc.vector.reciprocal(r[:], den[:])
            yy = smp.tile([P, W], F32, tag=f"yy{tag}", name=f"yy{tag}")
            nc.gpsimd.tensor_tensor(yy[:], Y[:], ay[:], mult)
            v = smp.tile([P, W], F32, tag=f"v{tag}", name=f"v{tag}")
            nc.gpsimd.tensor_tensor(v[:], yy[:], r[:], mult)
            Vn = smp.tile([P, W], F32, tag=f"V{tag}", name=f"V{tag}")
            nc.gpsimd.tensor_tensor(Vn[:], Vold_ap, v[:], add)
            return v, Vn

        def route_t1_iter(bg, it):
            a1t = a1t_of[bg]
            for h in range(2):
                if it == 2:
                    Vh = st["V_t1_1"][:, 8 * bg + 4 * h:8 * bg + 4 * h + 4]
                else:
                    Vh = st[("V_t1h", bg, it - 1, h)][:]
                Z = smp.tile([128, 4], F32, tag="Zt1", name="Zt1")
                Y = smp.tile([128, 4], F32, tag="Yt1", name="Yt1")
                for j in range(4):
                    bl = 4 * h + j
                    p_sl = a1t[:, bl * N:(bl + 1) * N]
                    zy_triple(p_sl, Vh[:, j:j + 1],
                              Z[:, j:j + 1], Y[:, j:j + 1])
                    if j % 2 == 1:
                        yield
                v, Vn = vstep(Vh, Z, Y, 128, 4, "t1")
                st[("V_t1h", bg, it, h)] = Vn
                if it == T:
                    nc.sync.dma_start(out1_d[bg][:, 4 * h:4 * h + 4], v[:])
                yield

        def route_a2_iter(bg, it):
            for blh in range(2):
                if it == 2:
                    Vh = st["Va2_1"][:, 2 * bg + blh:2 * bg + blh + 1]
                else:
                    Vh = st[("Va2h", bg, it - 1, blh)][:]
                Z = smp.tile([128, 1], F32, tag="Za2", name="Za2")
                Y = smp.tile([128, 1], F32, tag="Ya2", name="Ya2")
                off = (bg * 2 + blh) * N
                p_sl = a2t[:, off:off + N]
                zy_triple(p_sl, Vh[:, 0:1], Z[:, 0:1], Y[:, 0:1])
                v, Vn = vstep(Vh, Z, Y, 128, 1, "a2")
                st[("Va2h", bg, it, blh)] = Vn
                if it == T:
                    # out2[(q,ko2), (bg,blh)] device-native; host remaps
                    nc.sync.dma_start(
                        out2_d[:, 2 * bg + blh:2 * bg + blh + 1], v[:])
                yield

        # ---- emission schedule: diagonal (it2(bg) || it3(bg-1) || gen) ----
        st["m1ps"] = psM.tile([32, KO], F32, tag="m1ps", name="m1ps")

        def dbl(g):
            # two units per interleave round
            it = iter(g)
            while True:
                try:
                    next(it)
                except StopIteration:
                    return
                try:
                    next(it)
                except StopIteration:
                    pass
                yield

        _interleave(gen_phase(0))
        _interleave(m1_finalize())
        if T == 1:
            for bg in range(1, BG):
                _interleave(gen_phase(bg))
            for bg in range(BG):
                nc.sync.dma_start(out1_d[bg],
                                  st["V_t1_1"][:, 8 * bg:8 * bg + 8])
                nc.sync.dma_start(out2_d[:, 2 * bg:2 * bg + 2],
                                  st["Va2_1"][:, 2 * bg:2 * bg + 2])
        else:
            ph = 0
            while True:
                gens = []
                if ph + 1 < BG:
                    gens.append(dbl(gen_phase(ph + 1)))
                for bg in range(BG):
                    it = ph - bg + 2
                    if 2 <= it <= T:
                        gens.append(route_t1_iter(bg, it))
                        gens.append(route_a2_iter(bg, it))
                if not gens:
                    break
                _interleave(*gens)
                ph += 1

    nc.compile()
    return nc


def _get_program(T):
    if T not in _cache:
        _cache[T] = _build_program(T)
    return _cache[T]


# ----------------------------------------------------------------------------
# host-side output assembly
# ----------------------------------------------------------------------------
def _assemble(results):
    v = np.zeros((K, B, 1, 1, O), dtype=np.float32)
    for c, res in enumerate(results):
        o1 = res["out1"]          # [BG, 128=(16k+o), BL]
        o2 = res["out2"]          # [128=(q, 16(k-8)+o), 8=(bg,blh)]
        b0 = c * BSH
        # o1[bg, 16k+o, bl] -> v[k, b0+8bg+bl, 0, 0, o]
        t = o1.reshape(BG, 8, O, BL).transpose(1, 0, 3, 2)  # [k, bg, bl, o]
        v[:8, b0:b0 + BSH, 0, 0, :] = t.reshape(8, BSH, O)
        # o2[(q,16kk+o), (bg,blh)] -> v[8+kk, b0+8bg+4blh+q, 0, 0, o]
        t2 = o2.reshape(4, 2, O, BG, 2).transpose(1, 3, 4, 0, 2)
        v[8:, b0:b0 + BSH, 0, 0, :] = t2.reshape(2, BSH, O)
    return v


def run(x, routing_weights, num_iterations, trace=False):
    T = int(num_iterations)
    x = np.asarray(x, dtype=np.float32)
    w = np.asarray(routing_weights, dtype=np.float32)
    nc = _get_program(T)
    in_maps = _prep_core_inputs(x, w)
    kw = {}
    if trace:
        kw = dict(trace=True, trace_cores=list(range(NCORES)))
    res = run_bass_kernel_spmd(nc, in_maps, core_ids=list(range(NCORES)), **kw)
    return _assemble(res.results), res


def kernel(x, routing_weights, num_iterations):
    out, _ = run(x, routing_weights, num_iterations)
    return out
